# revision 22
# baseline (speedup 1.0000x reference)
"""DNC addressing kernel for Trainium2, 8 NeuronCores, batch-sharded.

Math reformulations vs the reference (numerically validated):
  * directional: the [B,N,N] shift kernel is circulant with row-constant
    normalization; dw[m] = sum_j gn[j] * w[(m-1024+j) % N] with j <= 15
    (Gaussian taps decay below f32 eps past j=6 even at max |sc|).
  * allocation: alloc[p] = exp(G_p + L_p), L = log1p(-u),
    G_p = sum over q with (u_q,q) lex-before (u_p,p) of L_q.
    Computed exactly with threshold-chunk sweeps:
      - earlier chunks use  u_q <= u_p   (value-equal earlier position counts)
      - later chunks use    u_q <  u_p
      - own chunk uses      u_q <  u_p   plus  (u_q == u_p) & (j < p)  via a
        constant strict-lower-triangular mask.

Layouts: "rm" means n = p*16 + c (contiguous 64B runs per partition, fast
DMA), "cm" means n = c*128 + p (forced for the allocation phase, whose
threshold chunks live on partitions). alloc converts cm->rm via a DRAM
round-trip through its own output tensor.
"""

import sys

for _p in ("/opt/trn_rl_repo", "/root/.axon_site/_ro/trn_rl_repo"):
    if _p not in sys.path:
        sys.path.append(_p)

import numpy as np

import concourse.bass as bass
import concourse.mybir as mybir
from bass_rust import AP
from concourse.tile import TileContext

F32 = mybir.dt.float32
AF = mybir.ActivationFunctionType
ALU = mybir.AluOpType
AX = mybir.AxisListType

NCORES = 8
B, N, W, C = 32, 2048, 64, 1024
BL = B // NCORES          # 4 rows per core
P = 128                   # partitions
NCH = N // P              # 16 chunks
KT = 16                   # directional taps
EPS = 1e-8

_CACHE = {}


def _split_waits(nc, cap=1):
    """Walrus codegen rejects instructions with more than ~1 semaphore wait
    (PE load-weights fails at 2). Hoist excess waits onto same-engine NOPs
    inserted just before the instruction."""
    import bass_rust

    wid = [0]
    for f in nc.m.functions:
        for blk in f.blocks:
            new = []
            for inst in blk.instructions:
                si = inst.sync_info
                waits = list(si.on_wait) if si is not None and si.on_wait else []
                if len(waits) > cap:
                    keep = waits[-cap:]
                    extra = waits[:-cap]
                    for i in range(0, len(extra), cap):
                        nop = bass_rust.InstNoOp(
                            name=f"WNOP-{wid[0]}", ins=[], outs=[])
                        wid[0] += 1
                        nop.engine = inst.engine
                        nop.sync_info = mybir.SyncInfo(
                            on_wait=extra[i:i + cap], on_update=[])
                        new.append(nop)
                    inst.sync_info = mybir.SyncInfo(
                        on_wait=keep, on_update=si.on_update)
                new.append(inst)
            blk.instructions[:] = new


def _win(ap, dims):
    """Raw windowed view of an SBUF tile AP: keep partition dim, replace the
    free dims (overlapping windows allowed)."""
    return AP(tensor=ap.tensor, offset=ap.offset, ap=[ap.ap[0]] + dims)


def _build():
    nc = bass.Bass()

    mem_d = nc.dram_tensor("mem", [BL, N, W], F32, kind="ExternalInput")
    coT_d = nc.dram_tensor("coT", [C, BL], F32, kind="ExternalInput")
    wcat_d = nc.dram_tensor("wcat", [C, 69], F32, kind="ExternalInput")
    bcat_d = nc.dram_tensor("bcat", [BL, 69], F32, kind="ExternalInput")
    wext_d = nc.dram_tensor("wext", [BL, N + KT - 1], F32, kind="ExternalInput")
    u_d = nc.dram_tensor("u", [BL, N], F32, kind="ExternalInput")
    tril_d = nc.dram_tensor("tril", [P, P], F32, kind="ExternalInput")
    ksqn_d = nc.dram_tensor("ksqn", [BL, KT], F32, kind="ExternalInput")

    o_ww = nc.dram_tensor("o_ww", [BL, N], F32, kind="ExternalOutput")
    o_cw = nc.dram_tensor("o_cw", [BL, N], F32, kind="ExternalOutput")
    o_dw = nc.dram_tensor("o_dw", [BL, N], F32, kind="ExternalOutput")
    o_al = nc.dram_tensor("o_al", [BL, N], F32, kind="ExternalOutput")

    kb_s = nc.dram_tensor("kb_s", [BL * W], F32, kind="Internal")
    gn_s = nc.dram_tensor("gn_s", [BL * KT], F32, kind="Internal")
    rs_s = nc.dram_tensor("rs_s", [BL], F32, kind="Internal")
    wh_s = nc.dram_tensor("wh_s", [BL], F32, kind="Internal")

    with TileContext(nc) as tc:
        with tc.tile_pool(name="sb", bufs=1) as pool, \
             tc.tile_pool(name="ps", bufs=2, space="PSUM") as ppool:

            dma = nc.sync.dma_start      # HWDGE engine 1
            dma2 = nc.scalar.dma_start   # HWDGE engine 2

            # ---------------- phase E loads first (critical path) ----------
            # u broadcast per row: the threshold sweeps read these.
            u_bs, L_bs, u_cms, L_cms = [], [], [], []
            for r in range(BL):
                u_b = pool.tile([P, N], F32, tag=f"u_b{r}")
                (dma if r % 2 == 0 else dma2)(
                    out=u_b[:], in_=AP(tensor=u_d, offset=r * N,
                                       ap=[[0, P], [1, N]]))
                u_bs.append(u_b)
                L_b = pool.tile([P, N], F32, tag=f"L_b{r}")
                nc.scalar.activation(L_b[:], u_b[:], AF.Ln, bias=1.0,
                                     scale=-1.0)
                L_bs.append(L_b)
                u_cm = pool.tile([P, NCH], F32, tag=f"u_cm{r}")
                (dma if r % 2 == 0 else dma2)(
                    out=u_cm[:], in_=AP(tensor=u_d, offset=r * N,
                                        ap=[[1, P], [P, NCH]]))
                u_cms.append(u_cm)
                L_cm = pool.tile([P, NCH], F32, tag=f"L_cm{r}")
                nc.scalar.activation(L_cm[:], u_cm[:], AF.Ln, bias=1.0,
                                     scale=-1.0)
                L_cms.append(L_cm)

            tril_sb = pool.tile([P, P], F32, tag="tril")
            dma2(out=tril_sb[:], in_=tril_d[:])


            # ---------------- phase E: allocation weights (cm layout) ------
            waste = pool.tile([P, N], F32, tag="waste")
            waste2 = pool.tile([P, P], F32, tag="waste2")
            al_rms = []
            for r in range(BL):
                u_b, L_b = u_bs[r], L_bs[r]
                u_cm, L_cm = u_cms[r], L_cms[r]
                gparts = pool.tile([P, NCH, 4], F32, tag=f"gp{r}")
                nc.vector.memset(gparts[:], 0.0)
                # TL_all[p, c, j] = tril[p, j] * L[c*128+j], all chunks at once
                TL_all = pool.tile([P, NCH, P], F32, tag="TLa")
                nc.vector.tensor_mul(
                    TL_all[:],
                    tril_sb[:].unsqueeze(1).broadcast_to([P, NCH, P]),
                    _win(L_b[:], [[P, NCH], [1, P]]))

                for c in range(NCH):
                    thr = u_cm[:, c:c + 1]
                    lo, hi = c * P, (c + 1) * P
                    if c > 0:
                        nc.vector.scalar_tensor_tensor(
                            out=waste[:, 0:lo], in0=u_b[:, 0:lo], scalar=thr,
                            in1=L_b[:, 0:lo], op0=ALU.is_le, op1=ALU.mult,
                            accum_out=gparts[:, c, 0:1])
                    if c < NCH - 1:
                        nc.vector.scalar_tensor_tensor(
                            out=waste[:, 0:N - hi], in0=u_b[:, hi:N],
                            scalar=thr, in1=L_b[:, hi:N], op0=ALU.is_lt,
                            op1=ALU.mult, accum_out=gparts[:, c, 1:2])
                    nc.vector.scalar_tensor_tensor(
                        out=waste2[:], in0=u_b[:, lo:hi], scalar=thr,
                        in1=L_b[:, lo:hi], op0=ALU.is_lt, op1=ALU.mult,
                        accum_out=gparts[:, c, 2:3])
                    nc.vector.scalar_tensor_tensor(
                        out=waste2[:], in0=u_b[:, lo:hi], scalar=thr,
                        in1=TL_all[:, c, :], op0=ALU.is_equal, op1=ALU.mult,
                        accum_out=gparts[:, c, 3:4])

                gsum = pool.tile([P, NCH], F32, tag=f"gsum{r}")
                nc.vector.tensor_reduce(gsum[:], gparts[:], axis=AX.X,
                                        op=ALU.add)
                gtot = pool.tile([P, NCH], F32, tag=f"gtot{r}")
                nc.vector.tensor_add(gtot[:], gsum[:], L_cm[:])
                al_cm = pool.tile([P, NCH], F32, tag=f"alcm{r}")
                nc.scalar.activation(al_cm[:], gtot[:], AF.Exp)
                # store (cm scatter) + read back in rm layout for combine
                dma(out=AP(tensor=o_al, offset=r * N,
                           ap=[[1, P], [P, NCH]]), in_=al_cm[:])
                al_rm = pool.tile([P, NCH], F32, tag=f"alrm{r}")
                dma(out=al_rm[:], in_=AP(tensor=o_al, offset=r * N,
                                         ap=[[NCH, P], [1, NCH]]))
                al_rms.append(al_rm)


            # ---------------- phase A: small matmuls + per-batch scalars ---
            coT_ld = pool.tile([P, C // P, BL], F32, tag="coT_ld")
            dma(out=coT_ld[:], in_=AP(tensor=coT_d, offset=0,
                                      ap=[[BL, P], [P * BL, C // P], [1, BL]]))
            wcat_ld = pool.tile([P, C // P, 69], F32, tag="wcat_ld")
            dma2(out=wcat_ld[:], in_=AP(tensor=wcat_d, offset=0,
                                        ap=[[69, P], [P * 69, C // P],
                                            [1, 69]]))
            # PE matmuls can carry only one sync wait; bounce operands
            # through DVE so they depend on a single semaphore.
            coT_sb = pool.tile([P, C // P, BL], F32, tag="coT")
            nc.vector.tensor_copy(coT_sb[:], coT_ld[:])
            wcat_sb = pool.tile([P, C // P, 69], F32, tag="wcat")
            nc.vector.tensor_copy(wcat_sb[:], wcat_ld[:])
            bcat_sb = pool.tile([BL, 69], F32, tag="bcat")
            dma(out=bcat_sb[:], in_=bcat_d[:])
            ksqn_sb = pool.tile([BL, KT], F32, tag="ksqn")
            dma(out=ksqn_sb[:], in_=ksqn_d[:])

            psA = ppool.tile([BL, 69], F32, tag="psA")
            for k in range(C // P):
                nc.tensor.matmul(psA[:], coT_sb[:, k, :], wcat_sb[:, k, :],
                                 start=(k == 0), stop=(k == C // P - 1))
            zs = pool.tile([BL, 69], F32, tag="zs")
            nc.vector.tensor_add(zs[:], psA[:], bcat_sb[:])

            kt_t = pool.tile([BL, W], F32, tag="kt")
            nc.scalar.activation(kt_t[:], zs[:, 0:W], AF.Tanh)
            # softplus via exp + ln(1+x): no Softplus act-table in this build
            bexp = pool.tile([BL, 1], F32, tag="bexp")
            nc.scalar.activation(bexp[:], zs[:, W:W + 1], AF.Exp)
            beta = pool.tile([BL, 1], F32, tag="beta")
            nc.scalar.activation(beta[:], bexp[:], AF.Ln, bias=1.0)
            kb = pool.tile([BL, W], F32, tag="kb")
            nc.vector.tensor_scalar_mul(kb[:], kt_t[:], beta[:])
            dma(out=kb_s[:].rearrange("(r w) -> r w", r=BL), in_=kb[:])

            z3 = zs[:, W + 1:W + 4]
            z3m = pool.tile([BL, 1], F32, tag="z3m")
            nc.vector.reduce_max(z3m[:], z3, axis=AX.X)
            nz3 = pool.tile([BL, 1], F32, tag="nz3")
            nc.scalar.mul(nz3[:], z3m[:], -1.0)
            e3 = pool.tile([BL, 3], F32, tag="e3")
            nc.scalar.activation(e3[:], z3, AF.Exp, bias=nz3[:])
            s3 = pool.tile([BL, 1], F32, tag="s3")
            nc.vector.reduce_sum(s3[:], e3[:], axis=AX.X)
            r3 = pool.tile([BL, 1], F32, tag="r3")
            nc.vector.reciprocal(r3[:], s3[:])
            scr = pool.tile([BL, 1], F32, tag="scr")
            nc.vector.tensor_sub(scr[:], e3[:, 2:3], e3[:, 0:1])
            sc = pool.tile([BL, 1], F32, tag="sc")
            nc.vector.tensor_mul(sc[:], scr[:], r3[:])
            sq = pool.tile([BL, 1], F32, tag="sq")
            nc.scalar.square(sq[:], sc[:])
            eps_t = pool.tile([BL, 1], F32, tag="eps")
            nc.vector.memset(eps_t[:], float(EPS))
            tau = pool.tile([BL, 1], F32, tag="tau")
            nc.scalar.activation(tau[:], sq[:], AF.Identity, bias=eps_t[:],
                                 scale=2.0)
            rtau = pool.tile([BL, 1], F32, tag="rtau")
            nc.vector.reciprocal(rtau[:], tau[:])
            garg = pool.tile([BL, KT], F32, tag="garg")
            nc.vector.tensor_scalar_mul(garg[:], ksqn_sb[:], rtau[:])
            g_t = pool.tile([BL, KT], F32, tag="g")
            nc.scalar.activation(g_t[:], garg[:], AF.Exp)
            S_t = pool.tile([BL, 1], F32, tag="S")
            nc.vector.reduce_sum(S_t[:], g_t[:], axis=AX.X)
            Se = pool.tile([BL, 1], F32, tag="Se")
            nc.scalar.activation(Se[:], S_t[:], AF.Identity, bias=eps_t[:])
            rS = pool.tile([BL, 1], F32, tag="rS")
            nc.vector.reciprocal(rS[:], Se[:])
            gn = pool.tile([BL, KT], F32, tag="gn")
            nc.vector.tensor_scalar_mul(gn[:], g_t[:], rS[:])
            dma(out=gn_s[:].rearrange("(r j) -> r j", r=BL), in_=gn[:])

            wgt = pool.tile([BL, 1], F32, tag="wgt")
            nc.scalar.activation(wgt[:], zs[:, W + 4:W + 5], AF.Sigmoid)
            wh = pool.tile([BL, 1], F32, tag="wh")
            nc.scalar.mul(wh[:], wgt[:], 0.5)
            dma(out=wh_s[:].rearrange("(r o) -> r o", r=BL), in_=wh[:])

            gnb = pool.tile([P, BL, KT], F32, tag="gnb")
            dma2(out=gnb[:], in_=AP(tensor=gn_s, offset=0,
                                    ap=[[0, P], [KT, BL], [1, KT]]))
            whb = pool.tile([P, BL], F32, tag="whb")
            dma2(out=whb[:], in_=AP(tensor=wh_s, offset=0,
                                    ap=[[0, P], [1, BL]]))
            ones_sb = pool.tile([P, 1], F32, tag="ones")
            nc.vector.memset(ones_sb[:], 1.0)

            # ---------------- phase B: sim = mem . (k*beta), rm layout -----
            # rm: n = p*16 + c; mem rows contiguous per partition (4KB).
            sim_all = pool.tile([P, BL, NCH], F32, tag="sim_all")
            for r in range(BL):
                memt = pool.tile([P, NCH, W], F32, tag=f"memt{r}")
                (dma if r % 2 == 0 else dma2)(
                    out=memt[:],
                    in_=AP(tensor=mem_d, offset=r * N * W,
                           ap=[[NCH * W, P], [W, NCH], [1, W]]))
                kb_b = pool.tile([P, W], F32, tag=f"kb_b{r}")
                (dma if r % 2 == 0 else dma2)(
                    out=kb_b[:], in_=AP(tensor=kb_s, offset=r * W,
                                        ap=[[0, P], [1, W]]))
                smul = pool.tile([P, NCH, W], F32, tag=f"smul{r}")
                nc.vector.tensor_mul(
                    smul[:], memt[:],
                    kb_b[:].unsqueeze(1).broadcast_to([P, NCH, W]))
                nc.vector.tensor_reduce(sim_all[:, r, :], smul[:], axis=AX.X,
                                        op=ALU.add)

            # ---------------- phase C: content softmax (no max-shift) -----
            e_cm = pool.tile([P, BL, NCH], F32, tag="e_cm")
            nc.scalar.activation(e_cm[:], sim_all[:], AF.Exp)
            esum = pool.tile([P, BL], F32, tag="esum")
            nc.vector.tensor_reduce(esum[:], e_cm[:], axis=AX.X, op=ALU.add)
            psC = ppool.tile([1, BL], F32, tag="psC")
            nc.tensor.matmul(psC[:], ones_sb[:], esum[:], start=True, stop=True)
            rCs = pool.tile([1, BL], F32, tag="rCs")
            nc.vector.reciprocal(rCs[:], psC[:])
            dma(out=rs_s[:].rearrange("(o r) -> o r", o=1), in_=rCs[:])
            rsb = pool.tile([P, BL], F32, tag="rsb")
            dma(out=rsb[:], in_=AP(tensor=rs_s, offset=0, ap=[[0, P], [1, BL]]))

            # ---------------- phase D: directional (16-tap), rm layout -----
            dw_all = pool.tile([P, BL, NCH], F32, tag="dw_all")
            for r in range(BL):
                vsb = pool.tile([P, NCH + KT - 1], F32, tag=f"vsb{r}")
                (dma if r % 2 == 0 else dma2)(
                    out=vsb[:], in_=AP(tensor=wext_d,
                                       offset=r * (N + KT - 1),
                                       ap=[[NCH, P], [1, NCH + KT - 1]]))
                dmul = pool.tile([P, NCH, KT], F32, tag=f"dmul{r}")
                nc.vector.tensor_mul(
                    dmul[:], _win(vsb[:], [[1, NCH], [1, KT]]),
                    gnb[:, r:r + 1, :].broadcast_to([P, NCH, KT]))
                nc.vector.tensor_reduce(dw_all[:, r, :], dmul[:], axis=AX.X,
                                        op=ALU.add)

            # ---------------- phase F: combine + store (rm layout) ---------
            rm_out = lambda d, r: AP(tensor=d, offset=r * N,
                                     ap=[[NCH, P], [1, NCH]])
            cw_rs = []
            for r in range(BL):
                cw_r = pool.tile([P, NCH], F32, tag=f"cw{r}")
                nc.vector.tensor_scalar_mul(cw_r[:], e_cm[:, r, :],
                                            rsb[:, r:r + 1])
                dma2(out=rm_out(o_cw, r), in_=cw_r[:])
                dma2(out=rm_out(o_dw, r), in_=dw_all[:, r, :])
                cw_rs.append(cw_r)
            for r in range(BL):
                dwal = pool.tile([P, NCH], F32, tag=f"dwal{r}")
                nc.vector.tensor_mul(dwal[:], dw_all[:, r, :], al_rms[r][:])
                tsum = pool.tile([P, NCH], F32, tag=f"tsum{r}")
                nc.vector.tensor_add(tsum[:], cw_rs[r][:], dwal[:])
                ww_r = pool.tile([P, NCH], F32, tag=f"ww{r}")
                nc.vector.tensor_scalar_mul(ww_r[:], tsum[:], whb[:, r:r + 1])
                dma2(out=rm_out(o_ww, r), in_=ww_r[:])

    _split_waits(nc)
    return nc


def _host_prep(inputs):
    co = np.ascontiguousarray(inputs["controller_output"], dtype=np.float32)
    prw = np.ascontiguousarray(inputs["prev_read_weights"], dtype=np.float32)
    memory = np.ascontiguousarray(inputs["memory"], dtype=np.float32)
    usage = np.ascontiguousarray(inputs["usage"], dtype=np.float32)

    wcat = np.concatenate([np.asarray(inputs["Wk"]), np.asarray(inputs["Wb"]),
                           np.asarray(inputs["Ws"]), np.asarray(inputs["Wg"])],
                          axis=0).T  # [C, 69]
    wcat = np.ascontiguousarray(wcat, dtype=np.float32)
    bcat = np.concatenate([np.asarray(inputs["bk"]), np.asarray(inputs["bb"]),
                           np.asarray(inputs["bs"]),
                           np.asarray(inputs["bg"])]).astype(np.float32)
    bcat_rep = np.ascontiguousarray(np.broadcast_to(bcat, (BL, 69)))

    # v[m] = w[(m-1024) % N]; extended with KT-1 wrap elements
    v = np.concatenate([prw[:, N // 2:], prw[:, :N // 2]], axis=1)
    wext = np.ascontiguousarray(
        np.concatenate([v, v[:, :KT - 1]], axis=1).astype(np.float32))

    tril = np.tril(np.ones((P, P), dtype=np.float32), k=-1)  # [p, j]: j < p
    ksqn = np.ascontiguousarray(np.broadcast_to(
        -(np.arange(KT, dtype=np.float32) ** 2), (BL, KT)), dtype=np.float32)

    in_maps = []
    for cidx in range(NCORES):
        rows = slice(cidx * BL, (cidx + 1) * BL)
        in_maps.append({
            "mem": np.ascontiguousarray(memory[rows]),
            "coT": np.ascontiguousarray(co[rows].T),
            "wcat": wcat,
            "bcat": bcat_rep,
            "wext": np.ascontiguousarray(wext[rows]),
            "u": np.ascontiguousarray(usage[rows]),
            "tril": tril,
            "ksqn": ksqn,
        })
    return in_maps


def kernel(**inputs):
    return _run(inputs, trace=False)[0]


def _run(inputs, trace=False):
    from concourse.bass_utils import run_bass_kernel_spmd

    if "nc" not in _CACHE:
        _CACHE["nc"] = _build()
    nc = _CACHE["nc"]

    in_maps = _host_prep(inputs)
    res = run_bass_kernel_spmd(nc, in_maps, core_ids=list(range(NCORES)),
                               trace=trace)

    ww = np.concatenate([res.results[i]["o_ww"] for i in range(NCORES)], axis=0)
    cw = np.concatenate([res.results[i]["o_cw"] for i in range(NCORES)], axis=0)
    dw = np.concatenate([res.results[i]["o_dw"] for i in range(NCORES)], axis=0)
    al = np.concatenate([res.results[i]["o_al"] for i in range(NCORES)], axis=0)
    out = (ww.astype(np.float32), cw.astype(np.float32),
           dw.astype(np.float32), al.astype(np.float32))
    return out, res
